# revision 5
# baseline (speedup 1.0000x reference)
"""Top-k masked cross-entropy (nn_GBCE) Bass kernel for 8 trn2 NeuronCores.

Problem: B=4096, V=50257, K=64, label_smoothing=0.1.
  truth = input[r, target[r]]; masked = input with target set to -inf;
  sel = [truth, top64(masked)]; loss = mean over rows of
  (0.9 * nll + 0.1 * smooth) on log_softmax(sel).

The loss needs only three per-row scalars: the truth logit v*, and the sum
T65 / exp-sum E65 / 65th value m65 of the raw row's top-65 (multiset
identity, ties included):
    a  = max(v*, m65);  S1 = T65 - a;  Z = exp(v*) + E65 - exp(a)
    per_ex = ln(Z) - (0.9 + 0.1/65)*v* - (0.1/65)*S1

Device algorithm per core (512 rows = 4 partition-blocks of 128):
  * Host casts the logits to fp16; each core streams its (512, 50257) slice
    in two ~25k-wide DMA tiles (51.5MB/core -- half the fp32 bytes; ~50KB
    per-row descriptors, far above the 4KB DMA saturation threshold). The
    final block of the last rep uses finer trailing tiles (TILES_FINE) so
    the end-of-stream DMA->fold serial drain is ~3.5us instead of ~14us.
  * Per tile, DVE folds pairwise with tensor_max 4x (fp16 packing -> 2
    elem/cyc), then max8 per 256-wide folded group (=4096 original cols)
    -> 112/120-wide candidate pool per row.
  * 9 rounds of max8 + match_replace extract the pool's top-72; ACT upcasts
    and accumulates T65/E65 (activation accum_out); closed form above, tail
    batched across the 4 blocks.
Sharding: 512 rows per core, data-parallel; host averages the 4096 per-row
losses (the only cross-core step).

Measured: rel err ~2.6e-4 vs fp32 reference (gate 2e-2); ~148us/exec in a
clean window, DMA-bound: a DMA-only variant of the same kernel measures
~146us = 353GB/s/core = 2.8TB/s aggregate (~97% of HBM). All engine work
(DVE folds/max8/extraction, ACT exp/ln) hides behind the stream. Queue
alternation (gpsimd+sync) and unroll>4 both measured WORSE; fp8 cannot help
(no 8-bit DVE packing modes). Shared-chip interference moves absolute
numbers +-7% between windows.

build(repeat=R, loop=True) wraps the body in a hardware For_i loop --
used by test.py to measure per-exec device time robustly under axon's
~70-90ms per-call dispatch jitter. The graded path (kernel()) uses
repeat=1 without the loop.
"""

import numpy as np

B = 4096
V = 50257
N_CORES = 8
ROWS_PER_CORE = B // N_CORES  # 512
N_BLOCKS = ROWS_PER_CORE // 128  # 4

NF = 4  # pairwise fold rounds per tile
GRPF = 256  # folded elems per max8 group (= 4096 original cols)
TILES = [25136, 25121]  # DMA tile widths (sum = V); ~50KB descriptors
assert sum(TILES) == V


def _pad(w):
    return ((w + (1 << NF) - 1) >> NF) << NF


TILES_FINE = [25136, 6280, 6280, 6280, 6281]  # last block of final rep:
# finer trailing tiles shrink the end-of-iteration DMA->fold drain
assert sum(TILES_FINE) == V


def _groups(tiles):
    return sum((_pad(w) >> NF) // GRPF + (1 if (_pad(w) >> NF) % GRPF else 0)
               for w in tiles)


N_GROUPS = _groups(TILES)  # 14
POOL_W = N_GROUPS * 8  # 112
N_GROUPS_F = _groups(TILES_FINE)  # 15
POOL_W_F = N_GROUPS_F * 8  # 120

NEG16 = -60000.0
C1 = float(0.9 + 0.1 / 65.0)
C2 = float(0.1 / 65.0)

_NC = None


def _body(nc, tc, x, tr, o, repeat=1, loop=False, unroll=4, mode="full", dma_alt=False, fine_last=True):
    import concourse.mybir as mybir

    f16 = mybir.dt.float16
    f32 = mybir.dt.float32
    Exp = mybir.ActivationFunctionType.Exp
    Ln = mybir.ActivationFunctionType.Ln
    Copy = mybir.ActivationFunctionType.Copy

    import contextlib

    with contextlib.ExitStack() as ctx:
        const = ctx.enter_context(tc.tile_pool(name="const", bufs=1))
        io = ctx.enter_context(tc.tile_pool(name="io", bufs=2))
        folds = ctx.enter_context(tc.tile_pool(name="folds", bufs=1))
        pools = ctx.enter_context(tc.tile_pool(name="pools", bufs=2))
        small = ctx.enter_context(tc.tile_pool(name="small", bufs=4))

        tr_t = const.tile([128, N_BLOCKS], f32, tag="tr")
        nc.sync.dma_start(out=tr_t, in_=tr)
        out_t = const.tile([128, N_BLOCKS], f32, tag="out")
        if mode == "dma":
            nc.vector.memset(out_t, 0.0)

        def emit_rep(fine=False):
            st_t65 = small.tile([128, N_BLOCKS], f32, tag="st_t65")
            st_e65 = small.tile([128, N_BLOCKS], f32, tag="st_e65")
            st_amax = small.tile([128, N_BLOCKS], f32, tag="st_amax")

            for blk in range(N_BLOCKS):
                r0 = blk * 128
                use_fine = fine and blk == N_BLOCKS - 1
                tiles_b = TILES_FINE if use_fine else TILES
                ng = N_GROUPS_F if use_fine else N_GROUPS
                pool_t = pools.tile([128, ng * 8], f16, tag="pool")
                g = 0
                c0 = 0
                for j, w in enumerate(tiles_b):
                    pw = _pad(w)
                    t = io.tile([128, pw], f16, tag="io")
                    if pw > w and mode != "dma":
                        nc.vector.memset(t[:, w:pw], NEG16)
                    if mode != "dve":
                        dq = nc.sync if (dma_alt and (blk * len(tiles_b) + j) % 2) else nc.gpsimd
                        dq.dma_start(
                            out=t[:, :w], in_=x[r0 : r0 + 128, c0 : c0 + w]
                        )
                    else:
                        # tiny DMA so tiles count as written; compute-bound timing
                        nc.gpsimd.dma_start(
                            out=t[:, :256], in_=x[r0 : r0 + 128, c0 : c0 + 256]
                        )
                    c0 += w
                    if mode == "dma":
                        continue
                    cur, cw = t, pw
                    for f in range(NF):
                        nxt = folds.tile([128, cw // 2], f16, tag=f"f{f}")
                        nc.vector.tensor_max(
                            out=nxt, in0=cur[:, : cw // 2], in1=cur[:, cw // 2 : cw]
                        )
                        cur, cw = nxt, cw // 2
                    for s in range(0, cw, GRPF):
                        sw = min(GRPF, cw - s)
                        nc.vector.max(
                            out=pool_t[:, g * 8 : g * 8 + 8], in_=cur[:, s : s + sw]
                        )
                        g += 1
                if mode == "dma":
                    continue
                assert g == ng, g

                top72 = small.tile([128, 72], f16, tag="top72")
                for r in range(9):
                    nc.vector.max(out=top72[:, r * 8 : r * 8 + 8], in_=pool_t)
                    if r < 8:
                        nc.vector.match_replace(
                            out=pool_t,
                            in_to_replace=top72[:, r * 8 : r * 8 + 8],
                            in_values=pool_t,
                            imm_value=NEG16,
                        )

                # T65 / E65 / m65 via ACT (fp16 -> fp32 with accumulate)
                t65f = small.tile([128, 65], f32, tag="t65f")
                nc.scalar.activation(
                    out=t65f,
                    in_=top72[:, :65],
                    func=Copy,
                    accum_out=st_t65[:, blk : blk + 1],
                )
                etmp = small.tile([128, 65], f32, tag="etmp")
                nc.scalar.activation(
                    out=etmp,
                    in_=top72[:, :65],
                    func=Exp,
                    accum_out=st_e65[:, blk : blk + 1],
                )
                nc.vector.tensor_max(
                    out=st_amax[:, blk : blk + 1],
                    in0=tr_t[:, blk : blk + 1],
                    in1=t65f[:, 64:65],
                )

            if mode == "dma":
                return

            # batched tail over [128, N_BLOCKS]
            expa = small.tile([128, N_BLOCKS], f32, tag="expa")
            nc.scalar.activation(out=expa, in_=st_amax, func=Exp)
            expv = small.tile([128, N_BLOCKS], f32, tag="expv")
            nc.scalar.activation(out=expv, in_=tr_t, func=Exp)
            z = small.tile([128, N_BLOCKS], f32, tag="z")
            nc.vector.tensor_add(out=z, in0=expv, in1=st_e65)
            nc.vector.tensor_sub(out=z, in0=z, in1=expa)
            lse = small.tile([128, N_BLOCKS], f32, tag="lse")
            nc.scalar.activation(out=lse, in_=z, func=Ln)

            # per_ex = lse - C1*v - C2*(t65 - amax)
            s1 = small.tile([128, N_BLOCKS], f32, tag="s1")
            nc.vector.tensor_sub(out=s1, in0=st_t65, in1=st_amax)
            nc.vector.tensor_scalar_mul(s1, s1, C2)
            sv = small.tile([128, N_BLOCKS], f32, tag="sv")
            nc.vector.tensor_scalar_mul(sv, tr_t, C1)
            nc.vector.tensor_sub(out=sv, in0=lse, in1=sv)
            nc.vector.tensor_sub(out=out_t, in0=sv, in1=s1)

        if loop and repeat > 1:
            assert repeat % unroll == 0
            with tc.For_i(0, repeat // unroll):
                for u in range(unroll):
                    emit_rep(fine=fine_last and u == unroll - 1)
        else:
            for r in range(repeat):
                emit_rep(fine=fine_last and r == repeat - 1)

        nc.sync.dma_start(out=o, in_=out_t)


def build(repeat=1, loop=False, unroll=4, mode="full", dma_alt=False, fine_last=True):
    global _NC
    key = (repeat, loop, unroll, mode, dma_alt, fine_last)
    if _NC is None or getattr(_NC, "_key", None) != key:
        import concourse.bacc as bacc
        import concourse.mybir as mybir
        from concourse.tile import TileContext

        nc = bacc.Bacc(
            "TRN2",
            debug=False,
            enable_asserts=False,
            num_devices=N_CORES,
        )
        x = nc.dram_tensor(
            "x", (ROWS_PER_CORE, V), mybir.dt.float16, kind="ExternalInput"
        )
        tr = nc.dram_tensor(
            "tr", (128, N_BLOCKS), mybir.dt.float32, kind="ExternalInput"
        )
        o = nc.dram_tensor(
            "o", (128, N_BLOCKS), mybir.dt.float32, kind="ExternalOutput"
        )
        with TileContext(nc) as tc:
            _body(
                nc, tc, x.ap(), tr.ap(), o.ap(),
                repeat=repeat, loop=loop, unroll=unroll, mode=mode,
                dma_alt=dma_alt, fine_last=fine_last,
            )
        nc.compile()
        nc._key = key
        nc._repeat = repeat
        _NC = nc
    return _NC


def make_in_maps(inp, tgt):
    truth = inp[np.arange(B), tgt].astype(np.float32)
    inp16 = inp.astype(np.float16)
    in_maps = []
    for k in range(N_CORES):
        sl = np.ascontiguousarray(inp16[k * ROWS_PER_CORE : (k + 1) * ROWS_PER_CORE])
        tb = np.ascontiguousarray(
            truth[k * ROWS_PER_CORE : (k + 1) * ROWS_PER_CORE]
            .reshape(N_BLOCKS, 128)
            .T
        )
        in_maps.append({"x": sl, "tr": tb})
    return in_maps


def gather_output(results):
    per = []
    for k in range(N_CORES):
        ob = np.asarray(results[k]["o"])  # (128, N_BLOCKS)
        per.append(ob.T.reshape(ROWS_PER_CORE))
    per_ex = np.concatenate(per)
    return np.float32(per_ex.mean(dtype=np.float64)), per_ex


def run(input, target, trace=False):
    from concourse import bass_utils

    inp = np.asarray(input, dtype=np.float32)
    tgt = np.asarray(target).astype(np.int64)
    nc = build()
    in_maps = make_in_maps(inp, tgt)
    res = bass_utils.run_bass_kernel_spmd(
        nc, in_maps, core_ids=list(range(N_CORES)), trace=trace
    )
    loss, per_ex = gather_output(res.results)
    return loss, per_ex, res


def kernel(input, target):
    loss, _, _ = run(input, target)
    return loss


# revision 6
# speedup vs baseline: 1.0039x; 1.0039x over previous
"""Top-k masked cross-entropy (nn_GBCE) Bass kernel for 8 trn2 NeuronCores.

Problem: B=4096, V=50257, K=64, label_smoothing=0.1.
  truth = input[r, target[r]]; masked = input with target set to -inf;
  sel = [truth, top64(masked)]; loss = mean over rows of
  (0.9 * nll + 0.1 * smooth) on log_softmax(sel).

The loss needs only three per-row scalars: the truth logit v*, and the sum
T65 / exp-sum E65 / 65th value m65 of the raw row's top-65 (multiset
identity, ties included):
    a  = max(v*, m65);  S1 = T65 - a;  Z = exp(v*) + E65 - exp(a)
    per_ex = ln(Z) - (0.9 + 0.1/65)*v* - (0.1/65)*S1

Device algorithm per core (512 rows = 4 partition-blocks of 128):
  * Host casts the logits to fp16; each core streams its (512, 50257) slice
    into two ~25k-wide SBUF buffers per block (51.5MB/core -- half the fp32
    bytes). Each buffer is filled by TWO half-width DMAs (~25KB descriptors):
    finer transfers pipeline measurably better across the 16 DMA engines
    (140.4us vs 145.7us dma-only) while the fold chain still runs once per
    buffer. The final block of the last rep uses finer trailing tiles
    (TILES_FINE) so the end-of-stream DMA->fold serial drain is ~3.5us
    instead of ~14us.
  * Per buffer, DVE folds pairwise with tensor_max 4x (fp16 packing -> 2
    elem/cyc), then max8 per 256-wide folded group (=4096 original cols)
    -> 112/120-wide candidate pool per row.
  * 9 rounds of max8 + match_replace extract the pool's top-72; ACT upcasts
    and accumulates T65/E65 (activation accum_out); closed form above, tail
    batched across the 4 blocks.
Sharding: 512 rows per core, data-parallel; host averages the 4096 per-row
losses (the only cross-core step).

Measured: rel err ~2.6e-4 vs fp32 reference (gate 2e-2); ~150us/exec
(window-dependent +-7%), DMA-bound at ~2.9TB/s aggregate HBM. All engine
work (DVE folds/max8/extraction, ACT exp/ln) hides behind the stream.
Measured dead ends: gpsimd+sync queue alternation (much worse), For_i
unroll 6/8 (worse), fp8 (no 8-bit DVE packing modes).

build(repeat=R, loop=True) wraps the body in a hardware For_i loop --
used by test.py to measure per-exec device time robustly under axon's
~70-90ms per-call dispatch jitter. The graded path (kernel()) uses
repeat=1 without the loop.
"""

import numpy as np

B = 4096
V = 50257
N_CORES = 8
ROWS_PER_CORE = B // N_CORES  # 512
N_BLOCKS = ROWS_PER_CORE // 128  # 4

NF = 4  # pairwise fold rounds per tile
GRPF = 256  # folded elems per max8 group (= 4096 original cols)
TILES = [25136, 25121]  # DMA tile widths (sum = V); ~50KB descriptors
assert sum(TILES) == V


def _pad(w):
    return ((w + (1 << NF) - 1) >> NF) << NF


TILES_FINE = [25136, 6280, 6280, 6280, 6281]  # last block of final rep:
# finer trailing tiles shrink the end-of-iteration DMA->fold drain
assert sum(TILES_FINE) == V


def _groups(tiles):
    return sum((_pad(w) >> NF) // GRPF + (1 if (_pad(w) >> NF) % GRPF else 0)
               for w in tiles)


N_GROUPS = _groups(TILES)  # 14
POOL_W = N_GROUPS * 8  # 112
N_GROUPS_F = _groups(TILES_FINE)  # 15
POOL_W_F = N_GROUPS_F * 8  # 120

NEG16 = -60000.0
C1 = float(0.9 + 0.1 / 65.0)
C2 = float(0.1 / 65.0)

_NC = None


def _body(nc, tc, x, tr, o, repeat=1, loop=False, unroll=4, mode="full", dma_alt=False, fine_last=True, dma_split=True):
    import concourse.mybir as mybir

    f16 = mybir.dt.float16
    f32 = mybir.dt.float32
    Exp = mybir.ActivationFunctionType.Exp
    Ln = mybir.ActivationFunctionType.Ln
    Copy = mybir.ActivationFunctionType.Copy

    import contextlib

    with contextlib.ExitStack() as ctx:
        const = ctx.enter_context(tc.tile_pool(name="const", bufs=1))
        io = ctx.enter_context(tc.tile_pool(name="io", bufs=2))
        folds = ctx.enter_context(tc.tile_pool(name="folds", bufs=1))
        pools = ctx.enter_context(tc.tile_pool(name="pools", bufs=2))
        small = ctx.enter_context(tc.tile_pool(name="small", bufs=4))

        tr_t = const.tile([128, N_BLOCKS], f32, tag="tr")
        nc.sync.dma_start(out=tr_t, in_=tr)
        out_t = const.tile([128, N_BLOCKS], f32, tag="out")
        if mode == "dma":
            nc.vector.memset(out_t, 0.0)

        def emit_rep(fine=False):
            st_t65 = small.tile([128, N_BLOCKS], f32, tag="st_t65")
            st_e65 = small.tile([128, N_BLOCKS], f32, tag="st_e65")
            st_amax = small.tile([128, N_BLOCKS], f32, tag="st_amax")

            for blk in range(N_BLOCKS):
                r0 = blk * 128
                use_fine = fine and blk == N_BLOCKS - 1
                tiles_b = TILES_FINE if use_fine else TILES
                ng = N_GROUPS_F if use_fine else N_GROUPS
                pool_t = pools.tile([128, ng * 8], f16, tag="pool")
                g = 0
                c0 = 0
                for j, w in enumerate(tiles_b):
                    pw = _pad(w)
                    t = io.tile([128, pw], f16, tag="io")
                    if pw > w and mode != "dma":
                        nc.vector.memset(t[:, w:pw], NEG16)
                    if mode != "dve":
                        # big buffers are filled by two half-width DMAs: finer
                        # transfers pipeline better across the 16 DMA engines
                        # (measured 140.4us vs 145.7us dma-only), while the
                        # fold chain still runs once over the whole buffer.
                        half = w // 2 if (dma_split and w > 16000) else w
                        for cs in range(0, w, half):
                            cwid = min(half, w - cs)
                            nc.gpsimd.dma_start(
                                out=t[:, cs : cs + cwid],
                                in_=x[r0 : r0 + 128, c0 + cs : c0 + cs + cwid],
                            )
                    else:
                        # tiny DMA so tiles count as written; compute-bound timing
                        nc.gpsimd.dma_start(
                            out=t[:, :256], in_=x[r0 : r0 + 128, c0 : c0 + 256]
                        )
                    c0 += w
                    if mode == "dma":
                        continue
                    cur, cw = t, pw
                    for f in range(NF):
                        nxt = folds.tile([128, cw // 2], f16, tag=f"f{f}")
                        nc.vector.tensor_max(
                            out=nxt, in0=cur[:, : cw // 2], in1=cur[:, cw // 2 : cw]
                        )
                        cur, cw = nxt, cw // 2
                    for s in range(0, cw, GRPF):
                        sw = min(GRPF, cw - s)
                        nc.vector.max(
                            out=pool_t[:, g * 8 : g * 8 + 8], in_=cur[:, s : s + sw]
                        )
                        g += 1
                if mode == "dma":
                    continue
                assert g == ng, g

                top72 = small.tile([128, 72], f16, tag="top72")
                for r in range(9):
                    nc.vector.max(out=top72[:, r * 8 : r * 8 + 8], in_=pool_t)
                    if r < 8:
                        nc.vector.match_replace(
                            out=pool_t,
                            in_to_replace=top72[:, r * 8 : r * 8 + 8],
                            in_values=pool_t,
                            imm_value=NEG16,
                        )

                # T65 / E65 / m65 via ACT (fp16 -> fp32 with accumulate)
                t65f = small.tile([128, 65], f32, tag="t65f")
                nc.scalar.activation(
                    out=t65f,
                    in_=top72[:, :65],
                    func=Copy,
                    accum_out=st_t65[:, blk : blk + 1],
                )
                etmp = small.tile([128, 65], f32, tag="etmp")
                nc.scalar.activation(
                    out=etmp,
                    in_=top72[:, :65],
                    func=Exp,
                    accum_out=st_e65[:, blk : blk + 1],
                )
                nc.vector.tensor_max(
                    out=st_amax[:, blk : blk + 1],
                    in0=tr_t[:, blk : blk + 1],
                    in1=t65f[:, 64:65],
                )

            if mode == "dma":
                return

            # batched tail over [128, N_BLOCKS]
            expa = small.tile([128, N_BLOCKS], f32, tag="expa")
            nc.scalar.activation(out=expa, in_=st_amax, func=Exp)
            expv = small.tile([128, N_BLOCKS], f32, tag="expv")
            nc.scalar.activation(out=expv, in_=tr_t, func=Exp)
            z = small.tile([128, N_BLOCKS], f32, tag="z")
            nc.vector.tensor_add(out=z, in0=expv, in1=st_e65)
            nc.vector.tensor_sub(out=z, in0=z, in1=expa)
            lse = small.tile([128, N_BLOCKS], f32, tag="lse")
            nc.scalar.activation(out=lse, in_=z, func=Ln)

            # per_ex = lse - C1*v - C2*(t65 - amax)
            s1 = small.tile([128, N_BLOCKS], f32, tag="s1")
            nc.vector.tensor_sub(out=s1, in0=st_t65, in1=st_amax)
            nc.vector.tensor_scalar_mul(s1, s1, C2)
            sv = small.tile([128, N_BLOCKS], f32, tag="sv")
            nc.vector.tensor_scalar_mul(sv, tr_t, C1)
            nc.vector.tensor_sub(out=sv, in0=lse, in1=sv)
            nc.vector.tensor_sub(out=out_t, in0=sv, in1=s1)

        if loop and repeat > 1:
            assert repeat % unroll == 0
            with tc.For_i(0, repeat // unroll):
                for u in range(unroll):
                    emit_rep(fine=fine_last and u == unroll - 1)
        else:
            for r in range(repeat):
                emit_rep(fine=fine_last and r == repeat - 1)

        nc.sync.dma_start(out=o, in_=out_t)


def build(repeat=1, loop=False, unroll=4, mode="full", dma_alt=False, fine_last=True, dma_split=True):
    global _NC
    key = (repeat, loop, unroll, mode, dma_alt, fine_last, dma_split)
    if _NC is None or getattr(_NC, "_key", None) != key:
        import concourse.bacc as bacc
        import concourse.mybir as mybir
        from concourse.tile import TileContext

        nc = bacc.Bacc(
            "TRN2",
            debug=False,
            enable_asserts=False,
            num_devices=N_CORES,
        )
        x = nc.dram_tensor(
            "x", (ROWS_PER_CORE, V), mybir.dt.float16, kind="ExternalInput"
        )
        tr = nc.dram_tensor(
            "tr", (128, N_BLOCKS), mybir.dt.float32, kind="ExternalInput"
        )
        o = nc.dram_tensor(
            "o", (128, N_BLOCKS), mybir.dt.float32, kind="ExternalOutput"
        )
        with TileContext(nc) as tc:
            _body(
                nc, tc, x.ap(), tr.ap(), o.ap(),
                repeat=repeat, loop=loop, unroll=unroll, mode=mode,
                dma_alt=dma_alt, fine_last=fine_last, dma_split=dma_split,
            )
        nc.compile()
        nc._key = key
        nc._repeat = repeat
        _NC = nc
    return _NC


def make_in_maps(inp, tgt):
    truth = inp[np.arange(B), tgt].astype(np.float32)
    inp16 = inp.astype(np.float16)
    in_maps = []
    for k in range(N_CORES):
        sl = np.ascontiguousarray(inp16[k * ROWS_PER_CORE : (k + 1) * ROWS_PER_CORE])
        tb = np.ascontiguousarray(
            truth[k * ROWS_PER_CORE : (k + 1) * ROWS_PER_CORE]
            .reshape(N_BLOCKS, 128)
            .T
        )
        in_maps.append({"x": sl, "tr": tb})
    return in_maps


def gather_output(results):
    per = []
    for k in range(N_CORES):
        ob = np.asarray(results[k]["o"])  # (128, N_BLOCKS)
        per.append(ob.T.reshape(ROWS_PER_CORE))
    per_ex = np.concatenate(per)
    return np.float32(per_ex.mean(dtype=np.float64)), per_ex


def run(input, target, trace=False):
    from concourse import bass_utils

    inp = np.asarray(input, dtype=np.float32)
    tgt = np.asarray(target).astype(np.int64)
    nc = build()
    in_maps = make_in_maps(inp, tgt)
    res = bass_utils.run_bass_kernel_spmd(
        nc, in_maps, core_ids=list(range(N_CORES)), trace=trace
    )
    loss, per_ex = gather_output(res.results)
    return loss, per_ex, res


def kernel(input, target):
    loss, _, _ = run(input, target)
    return loss
